# revision 1
# baseline (speedup 1.0000x reference)
"""DRL4EVRP pointer-network greedy decode on 8 Trainium2 NeuronCores.

Data-parallel: batch 512 is split into 8 shards of 64. Each core runs an
identical fully-unrolled 100-step decode:
  GRU cell -> attention (tanh + v-dot + softmax) -> context -> pointer logits
  -> argmax -> gather -> next decoder input.
Layouts: "Hp" = [h=128 partitions, (b, s) free]; "B" = [b=64 partitions, ...].
All arithmetic fp32 (argmax must track the fp32 reference to ~1e-6).
"""
import os
import numpy as np
from contextlib import ExitStack

import concourse.bacc as bacc
import concourse.tile as tile
from concourse import mybir
from concourse.bass_utils import run_bass_kernel_spmd

dt = mybir.dt
AF = mybir.ActivationFunctionType
ALU = mybir.AluOpType
AX = mybir.AxisListType

B, S, H = 512, 100, 128
NCORES = 8
BS = B // NCORES            # 64 batches per core
F = BS * S                  # 6400 free elements in Hp layout
NCH = 16                    # v-dot chunks per step
CH = F // NCH               # 400 columns per chunk
BIG = 1.0e4                 # tie-break offset (keeps iota integers exact in fp32)

# --- shared constant panel column layout (rows used vary per block) ---
_PANEL = {}
_pc = 0
def _pcol(name, rows, cols):
    global _pc
    _PANEL[name] = (_pc, rows, cols)
    _pc += cols
for _n, _r, _c in [
    ("W_sT", 2, H), ("W_dT", 2, H), ("W_decT", 2, H),
    ("Wa_sT", H, H), ("Wa_dT", H, H), ("Wa_hT", H, H),
    ("Wp_sT", H, H), ("Wp_cT", H, H),
    ("W_ihrT", H, H), ("W_ihzT", H, H), ("W_ihnT", H, H),
    ("W_hhrT", H, H), ("W_hhzT", H, H), ("W_hhnT", H, H),
    ("v_a", H, 1), ("v_p", H, 1),
    ("b_s", H, 1), ("b_d", H, 1), ("b_dec", H, 1),
    ("br_half", H, 1), ("bz_half", H, 1), ("b_in", H, 1), ("b_hn", H, 1),
    ("id64", 64, 64), ("iota", 64, S), ("iotaP", 64, S),
]:
    _pcol(_n, _r, _c)
PANEL_C = _pc

_CACHE = {}


def _build_program():
    DBG = os.environ.get("DEBUG_DUMP") == "1"
    nc = bacc.Bacc("TRN2")
    dbg = {}
    def dbg_out(name, shape, dty=dt.float32):
        dbg[name] = nc.dram_tensor("dbg_" + name, shape, dty, kind="ExternalOutput")

    panel_ext = nc.dram_tensor("panel", [H, PANEL_C], dt.float32, kind="ExternalInput")
    static_r_ext = nc.dram_tensor("static_r", [2, F], dt.float32, kind="ExternalInput")
    dynamic_r_ext = nc.dram_tensor("dynamic_r", [2, F], dt.float32, kind="ExternalInput")
    static_B_ext = nc.dram_tensor("static_B", [BS, 2 * S], dt.float32, kind="ExternalInput")
    idx_ext = nc.dram_tensor("tour_idx", [BS, S], dt.int32, kind="ExternalOutput")
    logp_ext = nc.dram_tensor("tour_logp", [BS, S], dt.float32, kind="ExternalOutput")

    with tile.TileContext(nc) as tc, ExitStack() as ctx:
        # ---------- pools ----------
        per = ctx.enter_context(tc.tile_pool(name="per", bufs=1))      # persistent
        big = ctx.enter_context(tc.tile_pool(name="big", bufs=1))      # [128, F] tensors
        sm = ctx.enter_context(tc.tile_pool(name="sm", bufs=2))        # small rotating
        wv = ctx.enter_context(tc.tile_pool(name="wv", bufs=2))        # wave staging
        ps_sc = ctx.enter_context(tc.tile_pool(name="ps_sc", bufs=2, space="PSUM"))
        ps_gr = ctx.enter_context(tc.tile_pool(name="ps_gr", bufs=2, space="PSUM"))
        ps_ms = ctx.enter_context(tc.tile_pool(name="ps_ms", bufs=2, space="PSUM"))
        ps_d = ctx.enter_context(tc.tile_pool(name="ps_d", bufs=1, space="PSUM"))

        # ---------- load inputs ----------
        panel = per.tile([H, PANEL_C], dt.float32, tag="panel")
        nc.sync.dma_start(panel[:], panel_ext.ap())

        def V(name):
            c0, rows, cols = _PANEL[name]
            return panel[:rows, c0:c0 + cols]

        static_Bt = per.tile([BS, 2 * S], dt.float32, tag="static_B")
        nc.sync.dma_start(static_Bt[:], static_B_ext.ap())

        # ---------- persistent state ----------
        A_fix = per.tile([H, F], dt.float32, tag="A_fix")
        P_fix = per.tile([H, F], dt.float32, tag="P_fix")
        G_T = per.tile([S, BS * H], dt.float32, tag="G_T")
        h_sb = per.tile([H, BS], dt.float32, tag="h")
        dec_sb = per.tile([2, BS], dt.float32, tag="dec")
        idx_st = per.tile([BS, S], dt.int32, tag="idx_st")
        cho_st = per.tile([BS, S], dt.float32, tag="cho_st")
        z_st = per.tile([BS, S], dt.float32, tag="z_st")

        # chunk plan for [H, F] matmul passes (N <= 512 per matmul)
        CHUNKS = [(i * 512, min(512, F - i * 512)) for i in range((F + 511) // 512)]

        # ---------- init: static_h / dynamic_h (their own slots, freed later via tags) ----------
        static_h = big.tile([H, F], dt.float32, tag="u0")
        dynamic_h = big.tile([H, F], dt.float32, tag="u1")
        with tc.tile_pool(name="init_in", bufs=2) as cin:
            inps = ps_sc
            for c0, n in CHUNKS:
                c_sb = cin.tile([2, 512], dt.float32, tag="cin")
                nc.sync.dma_start(c_sb[:, :n], static_r_ext.ap()[:, c0:c0 + n])
                p = inps.tile([H, 512], dt.float32, tag="p_sc")
                nc.tensor.matmul(p[:, :n], V("W_sT"), c_sb[:, :n],
                                 start=True, stop=True)
                nc.vector.tensor_scalar(static_h[:, c0:c0 + n], p[:, :n],
                                        V("b_s"), None, op0=ALU.add)
            for c0, n in CHUNKS:
                c_sb = cin.tile([2, 512], dt.float32, tag="cin")
                nc.sync.dma_start(c_sb[:, :n], dynamic_r_ext.ap()[:, c0:c0 + n])
                p = inps.tile([H, 512], dt.float32, tag="p_sc")
                nc.tensor.matmul(p[:, :n], V("W_dT"), c_sb[:, :n],
                                 start=True, stop=True)
                nc.vector.tensor_scalar(dynamic_h[:, c0:c0 + n], p[:, :n],
                                        V("b_d"), None, op0=ALU.add)
            # A_fix = Wa_s @ static_h + Wa_d @ dynamic_h
            for c0, n in CHUNKS:
                p = inps.tile([H, 512], dt.float32, tag="p_sc")
                nc.tensor.matmul(p[:, :n], V("Wa_sT"), static_h[:, c0:c0 + n],
                                 start=True, stop=False)
                nc.tensor.matmul(p[:, :n], V("Wa_dT"), dynamic_h[:, c0:c0 + n],
                                 start=False, stop=True)
                nc.vector.tensor_copy(A_fix[:, c0:c0 + n], p[:, :n])
            # P_fix = Wp_s @ static_h
            for c0, n in CHUNKS:
                p = inps.tile([H, 512], dt.float32, tag="p_sc")
                nc.tensor.matmul(p[:, :n], V("Wp_sT"), static_h[:, c0:c0 + n],
                                 start=True, stop=True)
                nc.vector.tensor_copy(P_fix[:, c0:c0 + n], p[:, :n])
            # G_T[s, b*H + h] = (Wp_c @ static_h_b)[h, s]
            for b in range(BS):
                p = inps.tile([S, H], dt.float32, tag="p_sc")
                nc.tensor.matmul(p[:], static_h[:, b * S:(b + 1) * S], V("Wp_cT"),
                                 start=True, stop=True)
                nc.vector.tensor_copy(G_T[:, b * H:(b + 1) * H], p[:])

        if DBG:
            for nm, shp in [("sh_c", [H, 512]), ("afix_c", [H, 512]),
                            ("pfix_c", [H, 512]), ("gt_c", [S, 512]),
                            ("x0", [H, BS]), ("h1", [H, BS]), ("w0", [H, BS]),
                            ("scores0", [BS, S]), ("attn0", [BS, S]),
                            ("d0", [H, BS]), ("logits0", [BS, S]),
                            ("decB0", [BS, 2]), ("dec1", [2, BS]),
                            ("uin_c", [H, 512]), ("tu_c", [H, 512]),
                            ("ein_c", [H, 512]), ("te_c", [H, 512])]:
                dbg_out(nm, shp)
            nc.sync.dma_start(dbg["sh_c"].ap(), static_h[:, :512])
            nc.sync.dma_start(dbg["afix_c"].ap(), A_fix[:, :512])
            nc.sync.dma_start(dbg["pfix_c"].ap(), P_fix[:, :512])
            nc.sync.dma_start(dbg["gt_c"].ap(), G_T[:, :512])

        # h0 = 0 ; dec0 = static[:, :, 0]
        nc.vector.memset(h_sb[:], 0.0)
        nc.sync.dma_start(dec_sb[:], static_r_ext.ap()[:, 0:F:S])

        BASES = [0, 32, 64, 96]

        HF = F // 2
        HB = BS // 2

        def dot_half(t_half, vname, out_B, half):
            """rows [half*32, half*32+32) of out_B <- repart(v.T @ t_half)."""
            for g in range(2):
                pw = ps_sc.tile([H, 512], dt.float32, tag="p_sc")
                for i, bp in enumerate(BASES):
                    c = g * 4 + i
                    nc.tensor.matmul(pw[bp:bp + 1, :CH], V(vname),
                                     t_half[:, c * CH:(c + 1) * CH],
                                     start=True, stop=True,
                                     tile_position=(0, bp) if bp == 96 else None)
                wave = wv.tile([H, CH], dt.float32, tag="wave")
                nc.vector.tensor_copy(wave[0:97, :], pw[0:97, :CH])
                row0 = half * 32 + g * 16
                nc.sync.dma_start(
                    out_B[row0:row0 + 16, :],
                    wave[0:97:32, :].rearrange("p (b s) -> p b s", b=4))

        # ---------- decode loop (fully unrolled) ----------
        for t in range(S):
            # GRU: x = W_dec @ dec + b_dec
            px = ps_gr.tile([H, BS], dt.float32, tag="p_gr")
            nc.tensor.matmul(px[:], V("W_decT"), dec_sb[:], start=True, stop=True)
            x_sb = sm.tile([H, BS], dt.float32, tag="x")
            nc.vector.tensor_scalar(x_sb[:], px[:], V("b_dec"), None, op0=ALU.add)

            # gates r, z: sigmoid(u) = 0.5*tanh(0.5*u) + 0.5
            pr = ps_gr.tile([H, BS], dt.float32, tag="p_gr")
            nc.tensor.matmul(pr[:], V("W_ihrT"), x_sb[:], start=True, stop=False)
            nc.tensor.matmul(pr[:], V("W_hhrT"), h_sb[:], start=False, stop=True)
            r_sb = sm.tile([H, BS], dt.float32, tag="r")
            nc.scalar.activation(r_sb[:], pr[:], AF.Tanh, bias=V("br_half"), scale=0.5)
            nc.vector.tensor_scalar(r_sb[:], r_sb[:], 0.5, 0.5, op0=ALU.mult, op1=ALU.add)

            pz = ps_gr.tile([H, BS], dt.float32, tag="p_gr")
            nc.tensor.matmul(pz[:], V("W_ihzT"), x_sb[:], start=True, stop=False)
            nc.tensor.matmul(pz[:], V("W_hhzT"), h_sb[:], start=False, stop=True)
            z_sb = sm.tile([H, BS], dt.float32, tag="z")
            nc.scalar.activation(z_sb[:], pz[:], AF.Tanh, bias=V("bz_half"), scale=0.5)
            nc.vector.tensor_scalar(z_sb[:], z_sb[:], 0.5, 0.5, op0=ALU.mult, op1=ALU.add)

            # n = tanh(i_n + b_in + r * (h_n + b_hn))
            pin = ps_gr.tile([H, BS], dt.float32, tag="p_gr")
            nc.tensor.matmul(pin[:], V("W_ihnT"), x_sb[:], start=True, stop=True)
            phn = ps_gr.tile([H, BS], dt.float32, tag="p_gr")
            nc.tensor.matmul(phn[:], V("W_hhnT"), h_sb[:], start=True, stop=True)
            hn_sb = sm.tile([H, BS], dt.float32, tag="hn")
            nc.vector.tensor_scalar(hn_sb[:], phn[:], V("b_hn"), None, op0=ALU.add)
            nc.vector.tensor_mul(hn_sb[:], r_sb[:], hn_sb[:])
            nc.vector.tensor_add(hn_sb[:], pin[:], hn_sb[:])
            n_sb = sm.tile([H, BS], dt.float32, tag="n")
            nc.scalar.activation(n_sb[:], hn_sb[:], AF.Tanh, bias=V("b_in"))

            # h' = n + z * (h - n)
            hm = sm.tile([H, BS], dt.float32, tag="hm")
            nc.vector.tensor_sub(hm[:], h_sb[:], n_sb[:])
            nc.vector.tensor_mul(hm[:], z_sb[:], hm[:])
            nc.vector.tensor_add(h_sb[:], n_sb[:], hm[:])

            # attention: w = Wa_h @ h'
            pwm = ps_ms.tile([H, BS], dt.float32, tag="p_ms")
            nc.tensor.matmul(pwm[:], V("Wa_hT"), h_sb[:], start=True, stop=True)
            w_sb = sm.tile([H, BS], dt.float32, tag="w")
            nc.vector.tensor_copy(w_sb[:], pwm[:])
            if DBG and t == 0:
                nc.sync.dma_start(dbg["x0"].ap(), x_sb[:])
                nc.sync.dma_start(dbg["h1"].ap(), h_sb[:])
                nc.sync.dma_start(dbg["w0"].ap(), w_sb[:])

            # U = A_fix + w (broadcast over s), split in halves on DVE/GPSIMD
            u0 = big.tile([H, HF], dt.float32, tag="u0")
            u1 = big.tile([H, HF], dt.float32, tag="u1")
            nc.vector.tensor_add(
                u0[:].rearrange("p (b s) -> p b s", b=HB),
                A_fix[:, :HF].rearrange("p (b s) -> p b s", b=HB),
                w_sb[:, :HB][:, :, None].to_broadcast((H, HB, S)))
            nc.gpsimd.tensor_add(
                u1[:].rearrange("p (b s) -> p b s", b=HB),
                A_fix[:, HF:].rearrange("p (b s) -> p b s", b=HB),
                w_sb[:, HB:][:, :, None].to_broadcast((H, HB, S)))
            t0 = big.tile([H, HF], dt.float32, tag="t0")
            t1 = big.tile([H, HF], dt.float32, tag="t1")
            nc.scalar.activation(t0[:], u0[:], AF.Tanh)
            nc.scalar.activation(t1[:], u1[:], AF.Tanh)

            scores_B = sm.tile([BS, S], dt.float32, tag="scores_B")
            dot_half(t0, "v_a", scores_B, 0)
            dot_half(t1, "v_a", scores_B, 1)
            if DBG and t == 0:
                nc.sync.dma_start(dbg["scores0"].ap(), scores_B[:])
            attn = sm.tile([BS, S], dt.float32, tag="attn")
            nc.scalar.activation(attn[:], scores_B[:], AF.Exp)
            z_t = sm.tile([BS, 1], dt.float32, tag="z_t")
            nc.vector.reduce_sum(z_t[:], attn[:], axis=AX.X)
            invz = sm.tile([BS, 1], dt.float32, tag="invz")
            nc.vector.reciprocal(invz[:], z_t[:])
            nc.vector.tensor_scalar(attn[:], attn[:], invz[:], None, op0=ALU.mult)

            # attnT via PE transpose
            pat = ps_ms.tile([S, BS], dt.float32, tag="p_ms")
            nc.tensor.transpose(pat[:], attn[:], V("id64"))
            attnT = sm.tile([S, BS], dt.float32, tag="attnT")
            nc.vector.tensor_copy(attnT[:], pat[:])

            # d = Wp_c @ context : 64 N=1 matmuls, one accumulation group
            pd0 = ps_d.tile([H, HB], dt.float32, tag="p_d0")
            pd1 = ps_d.tile([H, HB], dt.float32, tag="p_d1")
            for b in range(BS):
                tgt = pd0 if b < HB else pd1
                col = b % HB
                nc.tensor.matmul(tgt[:, col:col + 1], G_T[:, b * H:(b + 1) * H],
                                 attnT[:, b:b + 1], start=(col == 0),
                                 stop=(col == HB - 1), skip_group_check=True)
            d0 = sm.tile([H, HB], dt.float32, tag="d0")
            d1 = sm.tile([H, HB], dt.float32, tag="d1")
            nc.vector.tensor_copy(d0[:], pd0[:])
            nc.vector.tensor_copy(d1[:], pd1[:])
            if DBG and t == 0:
                nc.sync.dma_start(dbg["attn0"].ap(), attn[:])
                nc.sync.dma_start(dbg["d0"].ap()[:, :HB], d0[:])
                nc.sync.dma_start(dbg["d0"].ap()[:, HB:], d1[:])
                nc.sync.dma_start(dbg["uin_c"].ap(), u0[:, :512])
                nc.sync.dma_start(dbg["tu_c"].ap(), t0[:, :512])

            # pointer: E = P_fix + d (broadcast), halves on GPSIMD/DVE
            e0 = big.tile([H, HF], dt.float32, tag="u0")
            e1 = big.tile([H, HF], dt.float32, tag="u1")
            nc.gpsimd.tensor_add(
                e0[:].rearrange("p (b s) -> p b s", b=HB),
                P_fix[:, :HF].rearrange("p (b s) -> p b s", b=HB),
                d0[:, :, None].to_broadcast((H, HB, S)))
            nc.vector.tensor_add(
                e1[:].rearrange("p (b s) -> p b s", b=HB),
                P_fix[:, HF:].rearrange("p (b s) -> p b s", b=HB),
                d1[:, :, None].to_broadcast((H, HB, S)))
            te0 = big.tile([H, HF], dt.float32, tag="t0")
            te1 = big.tile([H, HF], dt.float32, tag="t1")
            nc.scalar.activation(te0[:], e0[:], AF.Tanh)
            nc.scalar.activation(te1[:], e1[:], AF.Tanh)

            logits_B = sm.tile([BS, S], dt.float32, tag="logits_B")
            dot_half(te0, "v_p", logits_B, 0)
            dot_half(te1, "v_p", logits_B, 1)

            if DBG and t == 0:
                nc.sync.dma_start(dbg["logits0"].ap(), logits_B[:])
                nc.sync.dma_start(dbg["ein_c"].ap(), e0[:, :512])
                nc.sync.dma_start(dbg["te_c"].ap(), te0[:, :512])
            # pointer softmax normalizer for logp
            pexp = sm.tile([BS, S], dt.float32, tag="pexp")
            nc.scalar.activation(pexp[:], logits_B[:], AF.Exp)
            nc.vector.reduce_sum(z_st[:, t:t + 1], pexp[:], axis=AX.X)

            # argmax (first-max tie break), chosen logit, Z for logp
            maxv = sm.tile([BS, 1], dt.float32, tag="maxv")
            nc.vector.reduce_max(maxv[:], logits_B[:], axis=AX.X)
            oh = sm.tile([BS, S], dt.float32, tag="oh")
            nc.vector.tensor_scalar(oh[:], logits_B[:], maxv[:], None, op0=ALU.is_equal)
            masked = sm.tile([BS, S], dt.float32, tag="masked")
            nc.vector.scalar_tensor_tensor(
                out=masked[:], in0=oh[:], scalar=-BIG, in1=V("iotaP"),
                op0=ALU.mult, op1=ALU.add)
            ptr = sm.tile([BS, 1], dt.float32, tag="ptr")
            nc.vector.tensor_reduce(ptr[:], masked[:], axis=AX.X, op=ALU.min)
            nc.vector.tensor_copy(idx_st[:, t:t + 1], ptr[:])
            oh1 = sm.tile([BS, S], dt.float32, tag="oh1")
            nc.vector.tensor_scalar(oh1[:], V("iota"), ptr[:], None, op0=ALU.is_equal)
            chm = sm.tile([BS, S], dt.float32, tag="chm")
            nc.vector.tensor_mul(chm[:], logits_B[:], oh1[:])
            nc.vector.reduce_sum(cho_st[:, t:t + 1], chm[:], axis=AX.X)

            # dec_new = static[b, :, ptr[b]]
            dg = sm.tile([BS, 2 * S], dt.float32, tag="dg")
            nc.vector.tensor_mul(
                dg[:].rearrange("p (c s) -> p c s", c=2),
                static_Bt[:].rearrange("p (c s) -> p c s", c=2),
                oh1[:, None, :].to_broadcast((BS, 2, S)))
            decB = sm.tile([BS, 2], dt.float32, tag="decB")
            nc.vector.tensor_reduce(decB[:], dg[:].rearrange("p (c s) -> p c s", c=2),
                                    axis=AX.X, op=ALU.add)
            pdc = ps_ms.tile([2, BS], dt.float32, tag="p_ms")
            nc.tensor.transpose(pdc[:], decB[:], V("id64"))
            nc.vector.tensor_copy(dec_sb[:], pdc[:])
            if DBG and t == 0:
                nc.sync.dma_start(dbg["decB0"].ap(), decB[:])
                nc.sync.dma_start(dbg["dec1"].ap(), dec_sb[:])

        # ---------- finalize: logp = chosen - ln(Z) ----------
        lnz = per.tile([BS, S], dt.float32, tag="lnz")
        nc.scalar.activation(lnz[:], z_st[:], AF.Ln)
        logp_sb = per.tile([BS, S], dt.float32, tag="logp")
        nc.vector.tensor_sub(logp_sb[:], cho_st[:], lnz[:])
        nc.sync.dma_start(logp_ext.ap(), logp_sb[:])
        nc.sync.dma_start(idx_ext.ap(), idx_st[:])

    nc.compile()
    return nc


def _host_prep(inputs):
    """Shared constant panel + per-core shards."""
    f32 = np.float32
    panel = np.zeros((H, PANEL_C), f32)

    def put(name, arr):
        c0, rows, cols = _PANEL[name]
        assert arr.shape == (rows, cols), (name, arr.shape)
        panel[:rows, c0:c0 + cols] = arr

    W_a = inputs["W_a"]
    W_p = inputs["W_p"]
    W_ih = inputs["W_ih"]
    W_hh = inputs["W_hh"]
    put("W_sT", inputs["W_s"].T)
    put("W_dT", inputs["W_d"].T)
    put("W_decT", inputs["W_dec"].T)
    put("Wa_sT", W_a[:, :H].T)
    put("Wa_dT", W_a[:, H:2 * H].T)
    put("Wa_hT", W_a[:, 2 * H:].T)
    put("Wp_sT", W_p[:, :H].T)
    put("Wp_cT", W_p[:, H:].T)
    put("W_ihrT", W_ih[0:H].T)
    put("W_ihzT", W_ih[H:2 * H].T)
    put("W_ihnT", W_ih[2 * H:].T)
    put("W_hhrT", W_hh[0:H].T)
    put("W_hhzT", W_hh[H:2 * H].T)
    put("W_hhnT", W_hh[2 * H:].T)
    put("v_a", inputs["v_a"][:, None])
    put("v_p", inputs["v_p"][:, None])
    put("b_s", inputs["b_s"][:, None])
    put("b_d", inputs["b_d"][:, None])
    put("b_dec", inputs["b_dec"][:, None])
    put("br_half", 0.5 * (inputs["b_ih"][0:H] + inputs["b_hh"][0:H])[:, None])
    put("bz_half", 0.5 * (inputs["b_ih"][H:2 * H] + inputs["b_hh"][H:2 * H])[:, None])
    put("b_in", inputs["b_ih"][2 * H:][:, None])
    put("b_hn", inputs["b_hh"][2 * H:][:, None])
    put("id64", np.eye(64, dtype=f32))
    iota = np.broadcast_to(np.arange(S, dtype=f32), (BS, S)).copy()
    put("iota", iota)
    put("iotaP", iota + np.float32(BIG))

    static = np.ascontiguousarray(inputs["static"], f32)
    dynamic = np.ascontiguousarray(inputs["dynamic"], f32)
    in_maps = []
    for c in range(NCORES):
        sl = slice(c * BS, (c + 1) * BS)
        st = static[sl]            # [64, 2, 100]
        dy = dynamic[sl]
        in_maps.append({
            "panel": panel,
            "static_r": np.ascontiguousarray(st.transpose(1, 0, 2).reshape(2, F)),
            "dynamic_r": np.ascontiguousarray(dy.transpose(1, 0, 2).reshape(2, F)),
            "static_B": np.ascontiguousarray(st.reshape(BS, 2 * S)),
        })
    return in_maps


def kernel(**inputs):
    if "nc" not in _CACHE:
        _CACHE["nc"] = _build_program()
    nc = _CACHE["nc"]
    in_maps = _host_prep(inputs)
    res = run_bass_kernel_spmd(nc, in_maps, list(range(NCORES)))
    _CACHE["last_result"] = res
    idx = np.concatenate([r["tour_idx"] for r in res.results], axis=0)
    logp = np.concatenate([r["tour_logp"] for r in res.results], axis=0)
    return idx, logp



# revision 2
# speedup vs baseline: 1.3626x; 1.3626x over previous
"""DRL4EVRP pointer-network greedy decode on 8 Trainium2 NeuronCores.

Data-parallel: batch 512 is split into 8 shards of 64. Each core runs an
identical fully-unrolled 100-step decode:
  GRU cell -> attention (tanh + v-dot + softmax) -> context -> pointer logits
  -> argmax -> gather -> next decoder input.
Layouts: "Hp" = [h=128 partitions, (b, s) free]; "B" = [b=64 partitions, ...].
All arithmetic fp32 (argmax must track the fp32 reference to ~1e-6).

Per-step pipeline is chunked by 16 batches (1600 cols) so DVE/GPSIMD adds,
ACT tanh and PE v-dots overlap; context is 64 M=1 matmuls packed 4-wide into
PE column strips (1-col weight loads) instead of 64 128-col weight loads.
"""
import os
import numpy as np
from contextlib import ExitStack

import concourse.bacc as bacc
import concourse.tile as tile
from concourse import mybir
from concourse.bass_utils import run_bass_kernel_spmd

dt = mybir.dt
AF = mybir.ActivationFunctionType
ALU = mybir.AluOpType
AX = mybir.AxisListType

B, S, H = 512, 100, 128
NCORES = 8
BS = B // NCORES            # 64 batches per core
F = BS * S                  # 6400 free elements in Hp layout
CK = 16                     # batches per pipeline chunk
CKC = CK * S                # 1600 cols per chunk
NCHK = BS // CK             # 4 chunks
BIG = 1.0e4                 # tie-break offset (keeps iota integers exact in fp32)

# --- shared constant panel column layout (rows used vary per block) ---
_PANEL = {}
_pc = 0
def _pcol(name, rows, cols):
    global _pc
    _PANEL[name] = (_pc, rows, cols)
    _pc += cols
for _n, _r, _c in [
    ("W_sT", 2, H), ("W_dT", 2, H), ("W_decT", 2, H),
    ("Wa_sT", H, H), ("Wa_dT", H, H), ("Wa_hT", H, H),
    ("Wp_sT", H, H), ("Wp_cT", H, H),
    ("W_ihrT", H, H), ("W_ihzT", H, H), ("W_ihnT", H, H),
    ("W_hhrT", H, H), ("W_hhzT", H, H), ("W_hhnT", H, H),
    ("v_a", H, 1), ("v_p", H, 1),
    ("b_s", H, 1), ("b_d", H, 1), ("b_dec", H, 1),
    ("br_half", H, 1), ("bz_half", H, 1), ("b_in", H, 1), ("b_hn", H, 1),
    ("id64", 64, 64), ("iota", 64, S), ("iotaP", 64, S),
]:
    _pcol(_n, _r, _c)
PANEL_C = _pc

_CACHE = {}


def _build_program():
    nc = bacc.Bacc("TRN2")

    panel_ext = nc.dram_tensor("panel", [H, PANEL_C], dt.float32, kind="ExternalInput")
    static_r_ext = nc.dram_tensor("static_r", [2, F], dt.float32, kind="ExternalInput")
    dynamic_r_ext = nc.dram_tensor("dynamic_r", [2, F], dt.float32, kind="ExternalInput")
    static_B_ext = nc.dram_tensor("static_B", [BS, 2 * S], dt.float32, kind="ExternalInput")
    idx_ext = nc.dram_tensor("tour_idx", [BS, S], dt.int32, kind="ExternalOutput")
    logp_ext = nc.dram_tensor("tour_logp", [BS, S], dt.float32, kind="ExternalOutput")

    with tile.TileContext(nc) as tc, ExitStack() as ctx:
        # ---------- pools ----------
        per = ctx.enter_context(tc.tile_pool(name="per", bufs=1))      # persistent
        big = ctx.enter_context(tc.tile_pool(name="big", bufs=1))      # [128, F] tensors
        sm = ctx.enter_context(tc.tile_pool(name="sm", bufs=2))        # small rotating
        wv = ctx.enter_context(tc.tile_pool(name="wv", bufs=2))        # dot wave staging
        wv2 = ctx.enter_context(tc.tile_pool(name="wv2", bufs=2))      # ctx wave staging
        ps_sc = ctx.enter_context(tc.tile_pool(name="ps_sc", bufs=2, space="PSUM"))
        ps_sm = ctx.enter_context(tc.tile_pool(name="ps_sm", bufs=2, space="PSUM"))
        ps_ctx = ctx.enter_context(tc.tile_pool(name="ps_ctx", bufs=4, space="PSUM"))

        # ---------- load inputs ----------
        panel = per.tile([H, PANEL_C], dt.float32, tag="panel")
        nc.sync.dma_start(panel[:], panel_ext.ap())

        def V(name):
            c0, rows, cols = _PANEL[name]
            return panel[:rows, c0:c0 + cols]

        static_Bt = per.tile([BS, 2 * S], dt.float32, tag="static_B")
        nc.sync.dma_start(static_Bt[:], static_B_ext.ap())

        # ---------- persistent state ----------
        A_fix = per.tile([H, F], dt.float32, tag="A_fix")
        P_fix = per.tile([H, F], dt.float32, tag="P_fix")
        G_T = per.tile([S, BS * H], dt.float32, tag="G_T")
        h_sb = per.tile([H, BS], dt.float32, tag="h")
        dec_sb = per.tile([2, BS], dt.float32, tag="dec")
        dT_sb = per.tile([BS, H], dt.float32, tag="dT")
        d_sb = per.tile([H, BS], dt.float32, tag="d")
        idx_st = per.tile([BS, S], dt.int32, tag="idx_st")
        cho_st = per.tile([BS, S], dt.float32, tag="cho_st")
        z_st = per.tile([BS, S], dt.float32, tag="z_st")

        # chunk plan for [H, F] matmul passes (N <= 512 per matmul)
        CHUNKS = [(i * 512, min(512, F - i * 512)) for i in range((F + 511) // 512)]

        # ---------- init: static_h / dynamic_h (slots reused by u/t later) ----------
        static_h = big.tile([H, F], dt.float32, tag="u_all")
        dynamic_h = big.tile([H, F], dt.float32, tag="t_all")
        with tc.tile_pool(name="init_in", bufs=2) as cin:
            inps = ps_sc
            for c0, n in CHUNKS:
                c_sb = cin.tile([2, 512], dt.float32, tag="cin")
                nc.sync.dma_start(c_sb[:, :n], static_r_ext.ap()[:, c0:c0 + n])
                p = inps.tile([H, 512], dt.float32, tag="p_sc")
                nc.tensor.matmul(p[:, :n], V("W_sT"), c_sb[:, :n],
                                 start=True, stop=True)
                nc.vector.tensor_scalar(static_h[:, c0:c0 + n], p[:, :n],
                                        V("b_s"), None, op0=ALU.add)
            for c0, n in CHUNKS:
                c_sb = cin.tile([2, 512], dt.float32, tag="cin")
                nc.sync.dma_start(c_sb[:, :n], dynamic_r_ext.ap()[:, c0:c0 + n])
                p = inps.tile([H, 512], dt.float32, tag="p_sc")
                nc.tensor.matmul(p[:, :n], V("W_dT"), c_sb[:, :n],
                                 start=True, stop=True)
                nc.vector.tensor_scalar(dynamic_h[:, c0:c0 + n], p[:, :n],
                                        V("b_d"), None, op0=ALU.add)
            # A_fix = Wa_s @ static_h + Wa_d @ dynamic_h
            for c0, n in CHUNKS:
                p = inps.tile([H, 512], dt.float32, tag="p_sc")
                nc.tensor.matmul(p[:, :n], V("Wa_sT"), static_h[:, c0:c0 + n],
                                 start=True, stop=False)
                nc.tensor.matmul(p[:, :n], V("Wa_dT"), dynamic_h[:, c0:c0 + n],
                                 start=False, stop=True)
                nc.vector.tensor_copy(A_fix[:, c0:c0 + n], p[:, :n])
            # P_fix = Wp_s @ static_h
            for c0, n in CHUNKS:
                p = inps.tile([H, 512], dt.float32, tag="p_sc")
                nc.tensor.matmul(p[:, :n], V("Wp_sT"), static_h[:, c0:c0 + n],
                                 start=True, stop=True)
                nc.vector.tensor_copy(P_fix[:, c0:c0 + n], p[:, :n])
            # G_T[s, b*H + h] = (Wp_c @ static_h_b)[h, s]
            for b in range(BS):
                p = inps.tile([S, H], dt.float32, tag="p_sc")
                nc.tensor.matmul(p[:], static_h[:, b * S:(b + 1) * S], V("Wp_cT"),
                                 start=True, stop=True)
                nc.vector.tensor_copy(G_T[:, b * H:(b + 1) * H], p[:])

        # h0 = 0 ; dec0 = static[:, :, 0]
        nc.vector.memset(h_sb[:], 0.0)
        nc.sync.dma_start(dec_sb[:], static_r_ext.ap()[:, 0:F:S])

        BASES = [0, 32, 64, 96]

        u_all = big.tile([H, F], dt.float32, tag="u_all")
        t_all = big.tile([H, F], dt.float32, tag="t_all")

        def fused_pass(fix, vec_sb, vname, out_B):
            """out_B[b, s] <- v . tanh(fix[:, b, s] + vec_sb[:, b]) chunked.

            Chunks 0,1 add on DVE, 2,3 on GPSIMD; tanh on ACT; v-dot as 4
            col-strip M=1 matmuls per chunk; PSUM->SBUF copy; strided DMA
            repartitions [4,4,100] -> 16 batch rows of out_B.
            """
            for k in range(NCHK):
                c0 = k * CKC
                u_ch = u_all[:, c0:c0 + CKC].rearrange("p (b s) -> p b s", b=CK)
                f_ch = fix[:, c0:c0 + CKC].rearrange("p (b s) -> p b s", b=CK)
                w_ch = vec_sb[:, k * CK:(k + 1) * CK][:, :, None].to_broadcast(
                    (H, CK, S))
                eng = nc.vector if k < 2 else nc.gpsimd
                eng.tensor_add(u_ch, f_ch, w_ch)
                nc.scalar.activation(t_all[:, c0:c0 + CKC],
                                     u_all[:, c0:c0 + CKC], AF.Tanh)
                pw = ps_sc.tile([H, 512], dt.float32, tag="p_sc")
                for i, bp in enumerate(BASES):
                    nc.tensor.matmul(
                        pw[bp:bp + 1, :400], vname,
                        t_all[:, c0 + i * 400:c0 + (i + 1) * 400],
                        start=True, stop=True,
                        tile_position=(0, bp) if bp == 96 else None)
                wave = wv.tile([H, 400], dt.float32, tag="wave")
                nc.vector.tensor_copy(wave[0:97, :], pw[0:97, :400])
                nc.sync.dma_start(
                    out_B[k * CK:(k + 1) * CK, :],
                    wave[0:97:32, :].rearrange("p (b s) -> p b s", b=4))

        # ---------- decode loop (fully unrolled) ----------
        for t in range(S):
            # GRU: x = W_dec @ dec + b_dec
            px = ps_sm.tile([H, BS], dt.float32, tag="p_sm")
            nc.tensor.matmul(px[:], V("W_decT"), dec_sb[:], start=True, stop=True)
            x_sb = sm.tile([H, BS], dt.float32, tag="x")
            nc.vector.tensor_scalar(x_sb[:], px[:], V("b_dec"), None, op0=ALU.add)

            # gates r, z: sigmoid(u) = 0.5*tanh(0.5*u) + 0.5
            pr = ps_sm.tile([H, BS], dt.float32, tag="p_sm")
            nc.tensor.matmul(pr[:], V("W_ihrT"), x_sb[:], start=True, stop=False)
            nc.tensor.matmul(pr[:], V("W_hhrT"), h_sb[:], start=False, stop=True)
            r_sb = sm.tile([H, BS], dt.float32, tag="r")
            nc.scalar.activation(r_sb[:], pr[:], AF.Tanh, bias=V("br_half"), scale=0.5)
            nc.vector.tensor_scalar(r_sb[:], r_sb[:], 0.5, 0.5, op0=ALU.mult, op1=ALU.add)

            pz = ps_sm.tile([H, BS], dt.float32, tag="p_sm")
            nc.tensor.matmul(pz[:], V("W_ihzT"), x_sb[:], start=True, stop=False)
            nc.tensor.matmul(pz[:], V("W_hhzT"), h_sb[:], start=False, stop=True)
            z_sb = sm.tile([H, BS], dt.float32, tag="z")
            nc.scalar.activation(z_sb[:], pz[:], AF.Tanh, bias=V("bz_half"), scale=0.5)
            nc.vector.tensor_scalar(z_sb[:], z_sb[:], 0.5, 0.5, op0=ALU.mult, op1=ALU.add)

            # n = tanh(i_n + b_in + r * (h_n + b_hn))
            pin = ps_sm.tile([H, BS], dt.float32, tag="p_sm")
            nc.tensor.matmul(pin[:], V("W_ihnT"), x_sb[:], start=True, stop=True)
            phn = ps_sm.tile([H, BS], dt.float32, tag="p_sm")
            nc.tensor.matmul(phn[:], V("W_hhnT"), h_sb[:], start=True, stop=True)
            hn_sb = sm.tile([H, BS], dt.float32, tag="hn")
            nc.vector.tensor_scalar(hn_sb[:], phn[:], V("b_hn"), None, op0=ALU.add)
            nc.vector.tensor_mul(hn_sb[:], r_sb[:], hn_sb[:])
            nc.vector.tensor_add(hn_sb[:], pin[:], hn_sb[:])
            n_sb = sm.tile([H, BS], dt.float32, tag="n")
            nc.scalar.activation(n_sb[:], hn_sb[:], AF.Tanh, bias=V("b_in"))

            # h' = n + z * (h - n)
            hm = sm.tile([H, BS], dt.float32, tag="hm")
            nc.vector.tensor_sub(hm[:], h_sb[:], n_sb[:])
            nc.vector.tensor_mul(hm[:], z_sb[:], hm[:])
            nc.vector.tensor_add(h_sb[:], n_sb[:], hm[:])

            # attention: w = Wa_h @ h'
            pwm = ps_sm.tile([H, BS], dt.float32, tag="p_sm")
            nc.tensor.matmul(pwm[:], V("Wa_hT"), h_sb[:], start=True, stop=True)
            w_sb = sm.tile([H, BS], dt.float32, tag="w")
            nc.vector.tensor_copy(w_sb[:], pwm[:])

            # scores = v_a . tanh(A_fix + w), pipelined by 16-batch chunks
            scores_B = sm.tile([BS, S], dt.float32, tag="scores_B")
            fused_pass(A_fix, w_sb, V("v_a"), scores_B)

            # softmax over s (B-layout)
            attn = sm.tile([BS, S], dt.float32, tag="attn")
            nc.scalar.activation(attn[:], scores_B[:], AF.Exp)
            z_t = sm.tile([BS, 1], dt.float32, tag="z_t")
            nc.vector.reduce_sum(z_t[:], attn[:], axis=AX.X)
            invz = sm.tile([BS, 1], dt.float32, tag="invz")
            nc.vector.reciprocal(invz[:], z_t[:])
            nc.vector.tensor_scalar(attn[:], attn[:], invz[:], None, op0=ALU.mult)

            # attnT via PE transpose
            pat = ps_sm.tile([S, BS], dt.float32, tag="p_sm")
            nc.tensor.transpose(pat[:], attn[:], V("id64"))
            attnT = sm.tile([S, BS], dt.float32, tag="attnT")
            nc.vector.tensor_copy(attnT[:], pat[:])

            # d[h, b] = sum_s G_T[s, b*H+h] * attnT[s, b]:
            # M=1 matmuls (1-col weight loads), 4 batches at a time in PE
            # column strips; bank m collects 16 batches as [4, 4*128] rows.
            for m in range(4):
                psc = ps_ctx.tile([H, 512], dt.float32, tag="p_ctx")
                for j in range(4):
                    for i, bp in enumerate(BASES):
                        b = 16 * m + 4 * i + j
                        nc.tensor.matmul(
                            psc[bp:bp + 1, j * H:(j + 1) * H],
                            attnT[:, b:b + 1], G_T[:, b * H:(b + 1) * H],
                            start=True, stop=True,
                            tile_position=(0, bp) if bp == 96 else None)
                w2 = wv2.tile([H, 512], dt.float32, tag="ctxw")
                nc.scalar.copy(w2[0:97, :], psc[0:97, :])
                nc.sync.dma_start(
                    dT_sb[16 * m:16 * m + 16, :],
                    w2[0:97:32, :].rearrange("p (j h) -> p j h", j=4))
            pd = ps_sm.tile([H, BS], dt.float32, tag="p_sm")
            nc.tensor.transpose(pd[:], dT_sb[:], V("id64"))
            nc.vector.tensor_copy(d_sb[:], pd[:])

            # logits = v_p . tanh(P_fix + d), same chunked pipeline
            logits_B = sm.tile([BS, S], dt.float32, tag="logits_B")
            fused_pass(P_fix, d_sb, V("v_p"), logits_B)

            # pointer softmax normalizer for logp
            pexp = sm.tile([BS, S], dt.float32, tag="pexp")
            nc.scalar.activation(pexp[:], logits_B[:], AF.Exp)
            nc.vector.reduce_sum(z_st[:, t:t + 1], pexp[:], axis=AX.X)

            # argmax (first-max tie break), chosen logit
            maxv = sm.tile([BS, 1], dt.float32, tag="maxv")
            nc.vector.reduce_max(maxv[:], logits_B[:], axis=AX.X)
            oh = sm.tile([BS, S], dt.float32, tag="oh")
            nc.vector.tensor_scalar(oh[:], logits_B[:], maxv[:], None, op0=ALU.is_equal)
            masked = sm.tile([BS, S], dt.float32, tag="masked")
            nc.vector.scalar_tensor_tensor(
                out=masked[:], in0=oh[:], scalar=-BIG, in1=V("iotaP"),
                op0=ALU.mult, op1=ALU.add)
            ptr = sm.tile([BS, 1], dt.float32, tag="ptr")
            nc.vector.tensor_reduce(ptr[:], masked[:], axis=AX.X, op=ALU.min)
            nc.vector.tensor_copy(idx_st[:, t:t + 1], ptr[:])
            oh1 = sm.tile([BS, S], dt.float32, tag="oh1")
            nc.vector.tensor_scalar(oh1[:], V("iota"), ptr[:], None, op0=ALU.is_equal)
            chm = sm.tile([BS, S], dt.float32, tag="chm")
            nc.vector.tensor_mul(chm[:], logits_B[:], oh1[:])
            nc.vector.reduce_sum(cho_st[:, t:t + 1], chm[:], axis=AX.X)

            # dec_new = static[b, :, ptr[b]]
            dg = sm.tile([BS, 2 * S], dt.float32, tag="dg")
            nc.vector.tensor_mul(
                dg[:].rearrange("p (c s) -> p c s", c=2),
                static_Bt[:].rearrange("p (c s) -> p c s", c=2),
                oh1[:, None, :].to_broadcast((BS, 2, S)))
            decB = sm.tile([BS, 2], dt.float32, tag="decB")
            nc.vector.tensor_reduce(decB[:], dg[:].rearrange("p (c s) -> p c s", c=2),
                                    axis=AX.X, op=ALU.add)
            pdc = ps_sm.tile([2, BS], dt.float32, tag="p_sm")
            nc.tensor.transpose(pdc[:], decB[:], V("id64"))
            nc.vector.tensor_copy(dec_sb[:], pdc[:])

        # ---------- finalize: logp = chosen - ln(Z) ----------
        lnz = per.tile([BS, S], dt.float32, tag="lnz")
        nc.scalar.activation(lnz[:], z_st[:], AF.Ln)
        logp_sb = per.tile([BS, S], dt.float32, tag="logp")
        nc.vector.tensor_sub(logp_sb[:], cho_st[:], lnz[:])
        nc.sync.dma_start(logp_ext.ap(), logp_sb[:])
        nc.sync.dma_start(idx_ext.ap(), idx_st[:])

    nc.compile()
    return nc


def _host_prep(inputs):
    """Shared constant panel + per-core shards."""
    f32 = np.float32
    panel = np.zeros((H, PANEL_C), f32)

    def put(name, arr):
        c0, rows, cols = _PANEL[name]
        assert arr.shape == (rows, cols), (name, arr.shape)
        panel[:rows, c0:c0 + cols] = arr

    W_a = inputs["W_a"]
    W_p = inputs["W_p"]
    W_ih = inputs["W_ih"]
    W_hh = inputs["W_hh"]
    put("W_sT", inputs["W_s"].T)
    put("W_dT", inputs["W_d"].T)
    put("W_decT", inputs["W_dec"].T)
    put("Wa_sT", W_a[:, :H].T)
    put("Wa_dT", W_a[:, H:2 * H].T)
    put("Wa_hT", W_a[:, 2 * H:].T)
    put("Wp_sT", W_p[:, :H].T)
    put("Wp_cT", W_p[:, H:].T)
    put("W_ihrT", W_ih[0:H].T)
    put("W_ihzT", W_ih[H:2 * H].T)
    put("W_ihnT", W_ih[2 * H:].T)
    put("W_hhrT", W_hh[0:H].T)
    put("W_hhzT", W_hh[H:2 * H].T)
    put("W_hhnT", W_hh[2 * H:].T)
    put("v_a", inputs["v_a"][:, None])
    put("v_p", inputs["v_p"][:, None])
    put("b_s", inputs["b_s"][:, None])
    put("b_d", inputs["b_d"][:, None])
    put("b_dec", inputs["b_dec"][:, None])
    put("br_half", 0.5 * (inputs["b_ih"][0:H] + inputs["b_hh"][0:H])[:, None])
    put("bz_half", 0.5 * (inputs["b_ih"][H:2 * H] + inputs["b_hh"][H:2 * H])[:, None])
    put("b_in", inputs["b_ih"][2 * H:][:, None])
    put("b_hn", inputs["b_hh"][2 * H:][:, None])
    put("id64", np.eye(64, dtype=f32))
    iota = np.broadcast_to(np.arange(S, dtype=f32), (BS, S)).copy()
    put("iota", iota)
    put("iotaP", iota + np.float32(BIG))

    static = np.ascontiguousarray(inputs["static"], f32)
    dynamic = np.ascontiguousarray(inputs["dynamic"], f32)
    in_maps = []
    for c in range(NCORES):
        sl = slice(c * BS, (c + 1) * BS)
        st = static[sl]            # [64, 2, 100]
        dy = dynamic[sl]
        in_maps.append({
            "panel": panel,
            "static_r": np.ascontiguousarray(st.transpose(1, 0, 2).reshape(2, F)),
            "dynamic_r": np.ascontiguousarray(dy.transpose(1, 0, 2).reshape(2, F)),
            "static_B": np.ascontiguousarray(st.reshape(BS, 2 * S)),
        })
    return in_maps


def kernel(**inputs):
    if "nc" not in _CACHE:
        _CACHE["nc"] = _build_program()
    nc = _CACHE["nc"]
    in_maps = _host_prep(inputs)
    res = run_bass_kernel_spmd(nc, in_maps, list(range(NCORES)))
    _CACHE["last_result"] = res
    idx = np.concatenate([r["tour_idx"] for r in res.results], axis=0)
    logp = np.concatenate([r["tour_logp"] for r in res.results], axis=0)
    return idx, logp
